# revision 13
# baseline (speedup 1.0000x reference)
"""Causal single-head attention (B=8, S=2048, E=768, H=64), v4.

One batch element per core. Single q-major score pass (max + exp with
natural per-partition bias), P transposed via paired xbar DMAs (Sync
engine is otherwise idle), AV with a ones-column in V giving softmax
row-sums for free, V projected x-stationary straight into [k, h] layout,
per-group drains, single-bank PSUM slot rings, PE warmup burst.
"""

import numpy as np
from contextlib import ExitStack

import concourse.bass as bass
import concourse.tile as tile
from concourse import bacc, mybir
from concourse.bass_utils import run_bass_kernel_spmd

F32 = mybir.dt.float32
F16 = mybir.dt.float16

B, S, E, H = 8, 2048, 768, 64
EC = E // 128
T = S // 128
NEG = -1.0e9


def build_attention_core():
    nc = bacc.Bacc(None, target_bir_lowering=False)
    xtbs = [
        nc.declare_dram_parameter(f"xt{b}", (128, EC, 512), F16, isOutput=False)
        for b in range(4)
    ]
    wqkv = nc.declare_dram_parameter("wqkv", (128, EC, 192), F16, isOutput=False)
    consts = nc.declare_dram_parameter("consts", (128, 3, 128), F32, isOutput=False)
    out = nc.declare_dram_parameter("out", (S, H), F32, isOutput=True)

    with ExitStack() as ctx:
        tc = ctx.enter_context(tile.TileContext(nc))
        singles = ctx.enter_context(tc.tile_pool(name="singles", bufs=1))
        # PSUM: oP 1 + s1P 7 = 8 banks; proj shares the s1 ring
        oP = ctx.enter_context(tc.tile_pool(name="oP", bufs=1, space="PSUM"))
        s1P = ctx.enter_context(tc.tile_pool(name="s1P", bufs=7, space="PSUM"))
        stats = ctx.enter_context(tc.tile_pool(name="stats", bufs=8))
        pPool = ctx.enter_context(tc.tile_pool(name="pPool", bufs=3))
        ptPool = ctx.enter_context(tc.tile_pool(name="ptPool", bufs=3))
        ofin = ctx.enter_context(tc.tile_pool(name="ofin", bufs=2))

        wqkv_sb = singles.tile([128, EC, 192], F16)
        consts_sb = singles.tile([128, 3, 128], F32)
        qt_sb = singles.tile([64, S], F16)
        kt_sb = singles.tile([64, S], F16)
        xt_bs = [singles.tile([128, EC, 512], F16, name=f"xt_sb{b}", tag=f"xt{b}")
                 for b in range(4)]

        nc.sync.dma_start(out=wqkv_sb[:], in_=wqkv[:])
        nc.sync.dma_start(out=xt_bs[0][:, 0, :], in_=xtbs[0][:, 0, :])
        nc.sync.dma_start(out=consts_sb[:], in_=consts[:])
        for c in range(1, EC):
            nc.gpsimd.dma_start(out=xt_bs[0][:, c, :], in_=xtbs[0][:, c, :])
        for b in range(1, 4):
            nc.gpsimd.dma_start(out=xt_bs[b][:], in_=xtbs[b][:])

        wqk_sb = wqkv_sb[:, :, 0:128]
        wv_sb = wqkv_sb[:, :, 128:192]
        mask_sb = consts_sb[:, 0, :]

        v_sb = singles.tile([128, T, H + 1], F16)
        nc.vector.memset(v_sb[:, :, H:H + 1], 1.0)

        # PE warmup while loads are in flight
        warm_sb = singles.tile([16, 32], F16)
        nc.vector.memset(warm_sb[:], 0.25)
        wm_ps = s1P.tile([128, 512], F32, tag="s1", name="wm_ps")
        for i in range(64):
            nc.tensor.matmul(
                wm_ps[0:32, 0:32], lhsT=warm_sb[:, 0:32], rhs=warm_sb[:, 0:32],
                start=True, stop=True,
            )
        wjunk = stats.tile([128, 1], F32, tag="wjunk")
        nc.vector.tensor_reduce(
            wjunk[0:32, :], wm_ps[0:32, 0:32],
            axis=mybir.AxisListType.X, op=mybir.AluOpType.max,
        )

        o_tiles = {}
        pair_state = {}

        def emit_proj_qk(b):
            cols = bass.ts(b, 512)
            qk_ps = s1P.tile([128, 512], F32, tag="s1", name="qk_ps")
            for c in range(EC):
                nc.tensor.matmul(
                    qk_ps[:], lhsT=wqk_sb[:, c, :], rhs=xt_bs[b][:, c, :],
                    start=(c == 0), stop=(c == EC - 1),
                )
            nc.scalar.copy(qt_sb[:, cols], qk_ps[0:64, :])
            nc.scalar.copy(kt_sb[:, cols], qk_ps[64:128, :])

        def emit_proj_v(b):
            v_ps = s1P.tile([128, 512], F32, tag="s1", name="v_ps")
            for jj in range(4):
                for c in range(EC):
                    nc.tensor.matmul(
                        v_ps[:, jj * H:(jj + 1) * H],
                        lhsT=xt_bs[b][:, c, jj * 128:(jj + 1) * 128],
                        rhs=wv_sb[:, c, :],
                        start=(c == 0), stop=(c == EC - 1),
                    )
            vp = v_ps
            v_view = bass.AP(
                tensor=vp.tensor, offset=vp.offset,
                ap=[vp.ap[0], [H, 4], [1, H]],
            )
            nc.vector.tensor_copy(v_sb[:, b * 4:(b + 1) * 4, 0:H], v_view)

        def tile_chunks(t):
            """returns (score-chunk thunks, finish thunk). Finish emits the
            final negm + all exps, so the caller can start the next tile's
            scores in between."""
            ki = (t + 1) * 128
            nblk = (ki + 511) // 512
            state = {"slots": []}

            def mk(bi, w, last):
                def run():
                    if "mx" not in state:
                        state["mx"] = stats.tile([128, 4], F32, tag="mx", name="mx")
                    mx = state["mx"]
                    s_t = s1P.tile([128, 512], F32, tag="s1")
                    state["slots"].append((s_t, w))
                    nc.tensor.matmul(
                        s_t[:, 0:w],
                        lhsT=qt_sb[:, bass.ts(t, 128)],
                        rhs=kt_sb[:, bi * 512:bi * 512 + w],
                        start=True, stop=True,
                    )
                    if last:
                        nc.vector.tensor_add(
                            s_t[:, w - 128:w], s_t[:, w - 128:w], mask_sb)
                    nc.vector.tensor_reduce(
                        mx[:, bi:bi + 1], s_t[:, 0:w],
                        axis=mybir.AxisListType.X, op=mybir.AluOpType.max,
                    )
                return run

            def finish():
                mx = state["mx"]
                negm = stats.tile([128, 1], F32, tag="negm", name="negm")
                nc.vector.tensor_reduce(
                    negm[:], mx[:, 0:nblk],
                    axis=mybir.AxisListType.X, op=mybir.AluOpType.max,
                    negate=True,
                )
                pair_state[t] = {
                    "pp": pPool.tile([128, 2048], F16, tag="pp", name="pp")
                }
                pp = pair_state[t]["pp"]
                for h, (s_h, w_h) in enumerate(state["slots"]):
                    nc.scalar.activation(
                        pp[:, h * 512:h * 512 + w_h],
                        s_h[:, 0:w_h],
                        mybir.ActivationFunctionType.Exp,
                        bias=negm[:], scale=1.0,
                    )

            chunks = [
                mk(bi, min(512, ki - bi * 512), bi == nblk - 1)
                for bi in range(nblk)
            ]
            return chunks, finish

        def emit_xbar(t):
            """transpose tile t's P: [128, ki] -> [128, t+1, 128]"""
            ki = (t + 1) * 128
            pt = ptPool.tile([128, 16, 128], F16, tag="pt", name="pt")
            pair_state[t]["pt"] = pt
            pp = pair_state[t]["pp"]
            nc.sync.dma_start(
                out=pt[:, 0:t + 1, :], in_=pp[:, 0:ki], transpose=True)

        def emit_av(i):
            g = i // 4
            if g not in o_tiles:
                o_tiles[g] = oP.tile([128, 512], F32, tag="o", name="o_g")
            o_g = o_tiles[g]
            pt = pair_state[i]["pt"]
            sl = (i % 4) * (H + 1)
            for j in range(i + 1):
                nc.tensor.matmul(
                    o_g[:, sl:sl + H + 1],
                    lhsT=pt[:, j, :],
                    rhs=v_sb[:, j, :],
                    start=(j == 0), stop=(j == i),
                )

        def emit_drain(g):
            o_ap = o_tiles[g][:]
            rs = stats.tile([128, 4], F32, tag="rs")
            sums_ap = bass.AP(
                tensor=o_ap.tensor, offset=o_ap.offset + H,
                ap=[o_ap.ap[0], [H + 1, 4], [0, 1]],
            )
            nc.vector.reciprocal(rs[:], sums_ap)
            of = ofin.tile([128, 4, H], F32, tag="of")
            o_data = bass.AP(
                tensor=o_ap.tensor, offset=o_ap.offset,
                ap=[o_ap.ap[0], [H + 1, 4], [1, H]],
            )
            rs_ap = rs[:]
            rs_b = bass.AP(
                tensor=rs_ap.tensor, offset=rs_ap.offset,
                ap=[rs_ap.ap[0], rs_ap.ap[1], [0, H]],
            )
            nc.vector.tensor_mul(of[:], o_data, rs_b)
            nc.gpsimd.dma_start(
                out=out.rearrange("(i p) h -> p i h", p=128)[:, 4 * g:4 * g + 4, :],
                in_=of[:],
            )

        # ---------------- schedule ----------------
        av_q = []   # (tile index, thunk)

        def pop_av(max_group=None, lag=2):
            while av_q and (
                (max_group is not None and av_q[0][0] // 4 <= max_group)
                or (max_group is None and len(av_q) > lag)
            ):
                av_q.pop(0)[1]()

        pending_fin = []

        for b in range(4):
            emit_proj_qk(b)
            emit_proj_v(b)
            if b >= 2:
                pop_av(max_group=b - 2)
                emit_drain(b - 2)
            for t in range(4 * b, 4 * b + 4):
                chunks, fin = tile_chunks(t)
                for ci, ch in enumerate(chunks):
                    ch()
                    if ci == 0 and pending_fin:
                        tprev, fprev = pending_fin.pop(0)
                        fprev()
                        emit_xbar(tprev)
                        av_q.append((tprev, lambda i=tprev: emit_av(i)))
                    pop_av(lag=2)
                pending_fin.append((t, fin))
        while pending_fin:
            tprev, fprev = pending_fin.pop(0)
            fprev()
            emit_xbar(tprev)
            av_q.append((tprev, lambda i=tprev: emit_av(i)))
        pop_av(max_group=2)
        emit_drain(2)
        pop_av(max_group=3)
        emit_drain(3)

    nc.finalize()
    return nc


_NC_CACHE = None


def make_in_maps(x, Wq, Wk, Wv):
    scale = np.sqrt(np.float32(E))
    wqk_np = np.concatenate([(Wq * scale).T, Wk.T], axis=1).astype(np.float16)
    wv_np = Wv.T.astype(np.float16)
    wqkv_np = np.concatenate(
        [wqk_np.reshape(EC, 128, 128), wv_np.reshape(EC, 128, H)], axis=2
    ).transpose(1, 0, 2).copy()
    mask_np = np.triu(np.full((128, 128), NEG, dtype=np.float32), k=1)
    consts_np = np.stack(
        [mask_np, np.ascontiguousarray(mask_np.T), np.eye(128, dtype=np.float32)],
        axis=1,
    ).astype(np.float32)
    maps = []
    for b in range(B):
        xt_b = np.ascontiguousarray(x[b].T).astype(np.float16)
        m = {"wqkv": wqkv_np, "consts": consts_np}
        for blk in range(4):
            m[f"xt{blk}"] = np.ascontiguousarray(
                xt_b.reshape(EC, 128, S)[:, :, blk * 512:(blk + 1) * 512]
                .transpose(1, 0, 2)
            )
        maps.append(m)
    return maps


def kernel(x: np.ndarray, Wq: np.ndarray, Wk: np.ndarray, Wv: np.ndarray) -> np.ndarray:
    global _NC_CACHE
    assert x.shape == (B, S, E)
    in_maps = make_in_maps(x, Wq, Wk, Wv)
    if _NC_CACHE is None:
        _NC_CACHE = build_attention_core()
    res = run_bass_kernel_spmd(_NC_CACHE, in_maps, core_ids=list(range(B)))
    return np.stack([res.results[b]["out"] for b in range(B)], axis=0)


if __name__ == "__main__":
    d = np.load("/tmp/ref_cache.npz")
    o = kernel(x=d["x"], Wq=d["Wq"], Wk=d["Wk"], Wv=d["Wv"])
    exp = d["expected"]
    rel = np.linalg.norm(o - exp) / np.linalg.norm(exp)
    print("Relative error:", rel)


# revision 14
# speedup vs baseline: 1.0717x; 1.0717x over previous
"""Causal single-head attention (B=8, S=2048, E=768, H=64), v4.

One batch element per core. Single q-major score pass (max + exp with
natural per-partition bias), P transposed via paired xbar DMAs (Sync
engine is otherwise idle), AV with a ones-column in V giving softmax
row-sums for free, V projected x-stationary straight into [k, h] layout,
per-group drains, single-bank PSUM slot rings, PE warmup burst.
"""

import numpy as np
from contextlib import ExitStack

import concourse.bass as bass
import concourse.tile as tile
from concourse import bacc, mybir
from concourse.bass_utils import run_bass_kernel_spmd

F32 = mybir.dt.float32
F16 = mybir.dt.float16

B, S, E, H = 8, 2048, 768, 64
EC = E // 128
T = S // 128
NEG = -1.0e9


def build_attention_core():
    nc = bacc.Bacc(None, target_bir_lowering=False)
    xtbs = [
        nc.declare_dram_parameter(f"xt{b}", (128, EC, 512), F16, isOutput=False)
        for b in range(4)
    ]
    wqkv = nc.declare_dram_parameter("wqkv", (128, EC, 192), F16, isOutput=False)
    consts = nc.declare_dram_parameter("consts", (128, 3, 128), F32, isOutput=False)
    out = nc.declare_dram_parameter("out", (S, H), F32, isOutput=True)

    with ExitStack() as ctx:
        tc = ctx.enter_context(tile.TileContext(nc))
        singles = ctx.enter_context(tc.tile_pool(name="singles", bufs=1))
        # PSUM: oP 1 + s1P 7 = 8 banks; proj shares the s1 ring
        oP = ctx.enter_context(tc.tile_pool(name="oP", bufs=1, space="PSUM"))
        s1P = ctx.enter_context(tc.tile_pool(name="s1P", bufs=7, space="PSUM"))
        stats = ctx.enter_context(tc.tile_pool(name="stats", bufs=8))
        pPool = ctx.enter_context(tc.tile_pool(name="pPool", bufs=3))
        ptPool = ctx.enter_context(tc.tile_pool(name="ptPool", bufs=3))
        ofin = ctx.enter_context(tc.tile_pool(name="ofin", bufs=2))

        wqkv_sb = singles.tile([128, EC, 192], F16)
        consts_sb = singles.tile([128, 3, 128], F32)
        qt_sb = singles.tile([64, S], F16)
        kt_sb = singles.tile([64, S], F16)
        xt_bs = [singles.tile([128, EC, 512], F16, name=f"xt_sb{b}", tag=f"xt{b}")
                 for b in range(4)]

        nc.sync.dma_start(out=wqkv_sb[:], in_=wqkv[:])
        nc.sync.dma_start(out=xt_bs[0][:, 0, :], in_=xtbs[0][:, 0, :])
        nc.sync.dma_start(out=consts_sb[:], in_=consts[:])
        for c in range(1, EC):
            nc.gpsimd.dma_start(out=xt_bs[0][:, c, :], in_=xtbs[0][:, c, :])
        for b in range(1, 4):
            nc.gpsimd.dma_start(out=xt_bs[b][:], in_=xtbs[b][:])

        wqk_sb = wqkv_sb[:, :, 0:128]
        wv_sb = wqkv_sb[:, :, 128:192]
        mask_sb = consts_sb[:, 0, :]

        v_sb = singles.tile([128, T, H + 1], F16)
        nc.vector.memset(v_sb[:, :, H:H + 1], 1.0)

        # PE warmup while loads are in flight
        warm_sb = singles.tile([16, 32], F16)
        nc.vector.memset(warm_sb[:], 0.25)
        wm_ps = s1P.tile([128, 512], F32, tag="s1", name="wm_ps")
        for i in range(64):
            nc.tensor.matmul(
                wm_ps[0:32, 0:32], lhsT=warm_sb[:, 0:32], rhs=warm_sb[:, 0:32],
                start=True, stop=True,
            )
        wjunk = stats.tile([128, 1], F32, tag="wjunk")
        nc.vector.tensor_reduce(
            wjunk[0:32, :], wm_ps[0:32, 0:32],
            axis=mybir.AxisListType.X, op=mybir.AluOpType.max,
        )

        o_tiles = {}
        pair_state = {}

        def emit_proj_qk(b):
            cols = bass.ts(b, 512)
            qk_ps = s1P.tile([128, 512], F32, tag="s1", name="qk_ps")
            for c in range(EC):
                nc.tensor.matmul(
                    qk_ps[:], lhsT=wqk_sb[:, c, :], rhs=xt_bs[b][:, c, :],
                    start=(c == 0), stop=(c == EC - 1),
                )
            nc.scalar.copy(qt_sb[:, cols], qk_ps[0:64, :])
            nc.scalar.copy(kt_sb[:, cols], qk_ps[64:128, :])

        def emit_proj_v(b):
            v_ps = s1P.tile([128, 512], F32, tag="s1", name="v_ps")
            for jj in range(4):
                for c in range(EC):
                    nc.tensor.matmul(
                        v_ps[:, jj * H:(jj + 1) * H],
                        lhsT=xt_bs[b][:, c, jj * 128:(jj + 1) * 128],
                        rhs=wv_sb[:, c, :],
                        start=(c == 0), stop=(c == EC - 1),
                    )
            vp = v_ps
            v_view = bass.AP(
                tensor=vp.tensor, offset=vp.offset,
                ap=[vp.ap[0], [H, 4], [1, H]],
            )
            nc.vector.tensor_copy(v_sb[:, b * 4:(b + 1) * 4, 0:H], v_view)

        def tile_chunks(t):
            """returns (score-chunk thunks, finish thunk). Finish emits the
            final negm + all exps, so the caller can start the next tile's
            scores in between."""
            ki = (t + 1) * 128
            nblk = (ki + 511) // 512
            state = {"slots": []}

            def mk(bi, w, last):
                def run():
                    if "mx" not in state:
                        state["mx"] = stats.tile([128, 4], F32, tag="mx", name="mx")
                    mx = state["mx"]
                    s_t = s1P.tile([128, 512], F32, tag="s1")
                    state["slots"].append((s_t, w))
                    nc.tensor.matmul(
                        s_t[:, 0:w],
                        lhsT=qt_sb[:, bass.ts(t, 128)],
                        rhs=kt_sb[:, bi * 512:bi * 512 + w],
                        start=True, stop=True,
                    )
                    if last:
                        nc.vector.tensor_add(
                            s_t[:, w - 128:w], s_t[:, w - 128:w], mask_sb)
                    nc.vector.tensor_reduce(
                        mx[:, bi:bi + 1], s_t[:, 0:w],
                        axis=mybir.AxisListType.X, op=mybir.AluOpType.max,
                    )
                return run

            def finish():
                mx = state["mx"]
                negm = stats.tile([128, 1], F32, tag="negm", name="negm")
                nc.vector.tensor_reduce(
                    negm[:], mx[:, 0:nblk],
                    axis=mybir.AxisListType.X, op=mybir.AluOpType.max,
                    negate=True,
                )
                pair_state[t] = {
                    "pp": pPool.tile([128, 2048], F16, tag="pp", name="pp")
                }
                pp = pair_state[t]["pp"]
                for h, (s_h, w_h) in enumerate(state["slots"]):
                    nc.scalar.activation(
                        pp[:, h * 512:h * 512 + w_h],
                        s_h[:, 0:w_h],
                        mybir.ActivationFunctionType.Exp,
                        bias=negm[:], scale=1.0,
                    )

            chunks = [
                mk(bi, min(512, ki - bi * 512), bi == nblk - 1)
                for bi in range(nblk)
            ]
            return chunks, finish

        def emit_xbar(t):
            """transpose tile t's P: [128, ki] -> [128, t+1, 128]"""
            ki = (t + 1) * 128
            pt = ptPool.tile([128, 16, 128], F16, tag="pt", name="pt")
            pair_state[t]["pt"] = pt
            pp = pair_state[t]["pp"]
            nc.sync.dma_start(
                out=pt[:, 0:t + 1, :], in_=pp[:, 0:ki], transpose=True)

        def emit_av(i):
            g = i // 4
            if g not in o_tiles:
                o_tiles[g] = oP.tile([128, 512], F32, tag="o", name="o_g")
            o_g = o_tiles[g]
            pt = pair_state[i]["pt"]
            sl = (i % 4) * (H + 1)
            for j in range(i + 1):
                nc.tensor.matmul(
                    o_g[:, sl:sl + H + 1],
                    lhsT=pt[:, j, :],
                    rhs=v_sb[:, j, :],
                    start=(j == 0), stop=(j == i),
                )

        def emit_drain(g):
            o_ap = o_tiles[g][:]
            rs = stats.tile([128, 4], F32, tag="rs")
            sums_ap = bass.AP(
                tensor=o_ap.tensor, offset=o_ap.offset + H,
                ap=[o_ap.ap[0], [H + 1, 4], [0, 1]],
            )
            nc.vector.reciprocal(rs[:], sums_ap)
            of = ofin.tile([128, 4, H], F32, tag="of")
            o_data = bass.AP(
                tensor=o_ap.tensor, offset=o_ap.offset,
                ap=[o_ap.ap[0], [H + 1, 4], [1, H]],
            )
            rs_ap = rs[:]
            rs_b = bass.AP(
                tensor=rs_ap.tensor, offset=rs_ap.offset,
                ap=[rs_ap.ap[0], rs_ap.ap[1], [0, H]],
            )
            nc.vector.tensor_mul(of[:], o_data, rs_b)
            nc.gpsimd.dma_start(
                out=out.rearrange("(i p) h -> p i h", p=128)[:, 4 * g:4 * g + 4, :],
                in_=of[:],
            )

        # ---------------- schedule ----------------
        av_q = []   # (tile index, thunk)

        def pop_av(max_group=None, lag=2):
            while av_q and (
                (max_group is not None and av_q[0][0] // 4 <= max_group)
                or (max_group is None and len(av_q) > lag)
            ):
                av_q.pop(0)[1]()

        pending_fin = []

        for b in range(4):
            emit_proj_qk(b)
            emit_proj_v(b)
            if b >= 2:
                pop_av(max_group=b - 2)
                emit_drain(b - 2)
            for t in range(4 * b + 3, 4 * b - 1, -1):
                chunks, fin = tile_chunks(t)
                for ci, ch in enumerate(chunks):
                    ch()
                    if ci == 0 and pending_fin:
                        tprev, fprev = pending_fin.pop(0)
                        fprev()
                        emit_xbar(tprev)
                        av_q.append((tprev, lambda i=tprev: emit_av(i)))
                    pop_av(lag=2)
                pending_fin.append((t, fin))
        while pending_fin:
            tprev, fprev = pending_fin.pop(0)
            fprev()
            emit_xbar(tprev)
            av_q.append((tprev, lambda i=tprev: emit_av(i)))
        pop_av(max_group=2)
        emit_drain(2)
        pop_av(max_group=3)
        emit_drain(3)

    nc.finalize()
    return nc


_NC_CACHE = None


def make_in_maps(x, Wq, Wk, Wv):
    scale = np.sqrt(np.float32(E))
    wqk_np = np.concatenate([(Wq * scale).T, Wk.T], axis=1).astype(np.float16)
    wv_np = Wv.T.astype(np.float16)
    wqkv_np = np.concatenate(
        [wqk_np.reshape(EC, 128, 128), wv_np.reshape(EC, 128, H)], axis=2
    ).transpose(1, 0, 2).copy()
    mask_np = np.triu(np.full((128, 128), NEG, dtype=np.float32), k=1)
    consts_np = np.stack(
        [mask_np, np.ascontiguousarray(mask_np.T), np.eye(128, dtype=np.float32)],
        axis=1,
    ).astype(np.float32)
    maps = []
    for b in range(B):
        xt_b = np.ascontiguousarray(x[b].T).astype(np.float16)
        m = {"wqkv": wqkv_np, "consts": consts_np}
        for blk in range(4):
            m[f"xt{blk}"] = np.ascontiguousarray(
                xt_b.reshape(EC, 128, S)[:, :, blk * 512:(blk + 1) * 512]
                .transpose(1, 0, 2)
            )
        maps.append(m)
    return maps


def kernel(x: np.ndarray, Wq: np.ndarray, Wk: np.ndarray, Wv: np.ndarray) -> np.ndarray:
    global _NC_CACHE
    assert x.shape == (B, S, E)
    in_maps = make_in_maps(x, Wq, Wk, Wv)
    if _NC_CACHE is None:
        _NC_CACHE = build_attention_core()
    res = run_bass_kernel_spmd(_NC_CACHE, in_maps, core_ids=list(range(B)))
    return np.stack([res.results[b]["out"] for b in range(B)], axis=0)


if __name__ == "__main__":
    d = np.load("/tmp/ref_cache.npz")
    o = kernel(x=d["x"], Wq=d["Wq"], Wk=d["Wk"], Wv=d["Wv"])
    exp = d["expected"]
    rel = np.linalg.norm(o - exp) / np.linalg.norm(exp)
    print("Relative error:", rel)
